# revision 45
# baseline (speedup 1.0000x reference)
"""ONI-Norm TRN2 kernel: all-fp16 PE path, streamed to the HBM roofline.

The v4 baseline ran every matmul in fp32 (4 cyc/row on the PE) and was
PE-bound at ~188us (PE 171us active). This version is DMA-bound (~122us
for 37.75 MB in+out per core at the sustained ~350 GB/s the HAM power
duty-cycle allows):
  - Z fp32->fp16 cast on ACT per chunk, with accum_out yielding the
    row-sum partials for free (no DVE reduces; GpSimd casts measured
    3.4ns/elem -- useless for bulk work).
  - PE transposes read fp16 (1 cyc/row), gram + Newton-Schulz +
    projection matmuls all fp16 with fp32 PSUM accumulation.
  - transpose PSUM->SBUF copies on DVE; mean correction of the gram and
    the projection bias/scale keep everything on uncentered Z so the
    whole pipeline streams chunk-by-chunk behind the input DMA.
  - 1024-wide projection epilogues ([128,1024] PSUM tiles) alternating
    ACT/DVE: amortizes the ~300ns fixed per-op cost that otherwise caps
    output supply at ~265 GB/s.
  - emission order == execution-time order everywhere (in-order engine
    queues; anything queued out of order head-of-line blocks its engine).
Numerics validated in sim_v5.py: rel_max ~1e-3 vs the 2e-2 gate.
"""

import math
from contextlib import ExitStack

import numpy as np

import concourse.bacc as bacc
import concourse.mybir as mybir
from concourse.bass import ds, ts, MemorySpace
from concourse.bass_isa import ReduceOp
from concourse.bass_utils import run_bass_kernel_spmd
from concourse.masks import make_identity
from concourse.tile import TileContext

P = 128
K = 18432
G_TOTAL = 16
N_CORES = 8
G_PER_CORE = G_TOTAL // N_CORES
ROWS_PER_CORE = G_PER_CORE * P
T_NS = 5
EPS = 1e-5
CHUNK = 2048
N_CHUNKS = K // CHUNK
SUB = 512
SUB_PER_CHUNK = CHUNK // SUB
N_SLICES = N_CHUNKS * SUB_PER_CHUNK   # 36 per group
F32 = mybir.dt.float32
F16 = mybir.dt.float16


def build_nc():
    nc = bacc.Bacc("TRN2", target_bir_lowering=False)
    x = nc.dram_tensor("x", [ROWS_PER_CORE, K], F32, kind="ExternalInput")
    y = nc.dram_tensor("y", [ROWS_PER_CORE, K], F32, kind="ExternalOutput")

    with TileContext(nc) as tc, ExitStack() as ctx:
        consts = ctx.enter_context(tc.tile_pool(name="consts", bufs=1))
        identity = consts.tile([P, P], F32)
        make_identity(nc, identity)
        identity16 = consts.tile([P, P], F16)
        make_identity(nc, identity16)
        eye_15 = consts.tile([P, P], F16)
        nc.vector.tensor_scalar_mul(eye_15, identity, 1.5)
        eps_eye = consts.tile([P, P], F32)
        nc.vector.tensor_scalar_mul(eps_eye, identity, EPS)
        ones = consts.tile([P, P], F32)
        nc.any.memset(ones, 1.0)

        # fp32 landing chunks (rotating) and persistent fp16 copies
        zfp = ctx.enter_context(tc.tile_pool(name="zf", bufs=6))
        zpool = ctx.enter_context(tc.tile_pool(name="z", bufs=G_PER_CORE * N_CHUNKS))
        ztp = ctx.enter_context(tc.tile_pool(name="zt", bufs=3))
        outp = ctx.enter_context(tc.tile_pool(name="out", bufs=8))
        nsp = ctx.enter_context(tc.tile_pool(name="ns", bufs=2))
        vecp = ctx.enter_context(tc.tile_pool(name="vec", bufs=2))
        # PSUM banks: S(1) + tp(2) + pr(2x2 for 1024-wide tiles) + nv(1) = 8
        ps_S = ctx.enter_context(tc.tile_pool(name="psS", bufs=1, space=MemorySpace.PSUM))
        ps_tp = ctx.enter_context(tc.tile_pool(name="psT", bufs=2, space=MemorySpace.PSUM))
        ps_pr = ctx.enter_context(tc.tile_pool(name="psP", bufs=2, space=MemorySpace.PSUM))
        ps_nv = ctx.enter_context(tc.tile_pool(name="psN", bufs=1, space=MemorySpace.PSUM))
        ps_ns = ps_nv
        ps_vec = ps_nv

        st = [dict() for _ in range(G_PER_CORE)]

        # sub-split chunks: first of g0 (PE warm-up). A last-chunk split for
        # g1 was tried and measured neutral: out1's start is bus-FIFO and
        # HAM-window bound, not gated by the g1 cast->mean->NS chain.
        SPLITS = {0: (0,), 1: ()}

        def emit_dma(g):
            # all input DMA triggers upfront on the SP queue
            s = st[g]
            s["zs"] = []
            s["zf"] = []
            n_acc = N_CHUNKS + 3 * len(SPLITS[g])
            s["rsum_parts"] = vecp.tile([P, n_acc], F32, name=f"rsp{g}")
            for c in range(N_CHUNKS):
                zf = zfp.tile([P, CHUNK], F32, tag="zf", name=f"zf{g}_{c}")
                z16 = zpool.tile([P, CHUNK], F16, tag="z", name=f"z{g}_{c}")
                if c in SPLITS[g]:
                    for t4 in range(SUB_PER_CHUNK):
                        nc.sync.dma_start(
                            zf[:, ts(t4, SUB)],
                            x[ds(g * P, P), ds(c * CHUNK + t4 * SUB, SUB)],
                        )
                else:
                    nc.sync.dma_start(zf, x[ds(g * P, P), ts(c, CHUNK)])
                s["zf"].append(zf)
                s["zs"].append(z16)

        def emit_cast(g, c):
            # fp32->fp16 cast + fused row-sum partial in one ACT pass.
            # Emitted chunk-by-chunk so nothing with unresolved deps ever
            # sits ahead of a cast in the in-order ACT queue.
            s = st[g]
            zf, z16 = s["zf"][c], s["zs"][c]
            acc = c + 3 * sum(1 for sc in SPLITS[g] if sc < c)
            if c in SPLITS[g]:
                for t4 in range(SUB_PER_CHUNK):
                    nc.scalar.activation(
                        z16[:, ts(t4, SUB)], zf[:, ts(t4, SUB)],
                        mybir.ActivationFunctionType.Identity,
                        accum_out=s["rsum_parts"][:, ds(acc + t4, 1)],
                    )
            else:
                nc.scalar.activation(
                    z16, zf, mybir.ActivationFunctionType.Identity,
                    accum_out=s["rsum_parts"][:, ds(acc, 1)],
                )

        # gram works in 1024-wide big-slices: 8 transposes land in one
        # [128,1024] fp16 PSUM tile (exactly one bank), ONE DVE copy moves
        # it to SBUF (half the per-op fixed cost + semaphores of 512-wide)
        BIG = 2 * SUB
        N_BSLICES = K // BIG  # 18 per group, 2 per chunk

        def emit_gram_T(g, bsi):
            s = st[g]
            c, h = divmod(bsi, 2)
            tp = ps_tp.tile([P, BIG], F16, tag="tp", name=f"tp{g}_{bsi}")
            for b in range(BIG // P):
                nc.tensor.transpose(
                    tp[:, ts(b, P)],
                    s["zs"][c][:, ds(h * BIG + b * P, P)],
                    identity16,
                )
            zt = ztp.tile([P, BIG], F16, tag="zt", name=f"zt{g}_{bsi}")
            nc.vector.tensor_copy(zt, tp)
            s.setdefault("zt_pend", {})[bsi] = zt

        def emit_gram_M(g, bsi):
            s = st[g]
            if bsi == 0:
                s["S_ps"] = ps_S.tile([P, P], F32, tag="S", name=f"Sps{g}")
            zt = s["zt_pend"].pop(bsi)
            for b in range(BIG // P):
                nc.tensor.matmul(
                    s["S_ps"], zt[:, ts(b, P)], zt[:, ts(b, P)],
                    start=(bsi == 0 and b == 0), stop=False,
                )

        def emit_gram_slice(g, bsi):
            # transposes of big-slice bsi, then matmuls of bsi-1 (1 lag)
            emit_gram_T(g, bsi)
            if bsi > 0:
                emit_gram_M(g, bsi - 1)
            if bsi == N_BSLICES - 1:
                emit_gram_M(g, bsi)

        def emit_mean_chain(g):
            s = st[g]
            rsum = vecp.tile([P, 1], F32, name=f"rs{g}")
            nc.vector.tensor_reduce(
                rsum, s["rsum_parts"], mybir.AxisListType.X, mybir.AluOpType.add
            )
            mean = vecp.tile([P, 1], F32, name=f"mean{g}")
            nc.vector.tensor_scalar_mul(mean, rsum, 1.0 / K)
            mean16 = vecp.tile([P, 1], F16, name=f"mean16_{g}")
            nc.vector.tensor_copy(mean16, mean)
            s["mean16"] = mean16
            m12 = vecp.tile([P, 1], F32, name=f"m12{g}")
            nc.vector.tensor_scalar_mul(m12, rsum, math.sqrt(K / P) / K)
            Mm = vecp.tile([P, P], F32, name=f"Mm{g}")
            nc.vector.tensor_scalar_mul(Mm, ones, m12)
            M_ps = ps_vec.tile([P, P], F32, tag="nv", name=f"Mps{g}")
            nc.tensor.matmul(M_ps, Mm, identity, start=True, stop=True)
            M128a = vecp.tile([P, P], F32, name=f"Ma{g}")
            nc.vector.tensor_copy(M128a, M_ps)
            M128b = vecp.tile([P, P], F32, name=f"Mb{g}")
            nc.vector.tensor_scalar_mul(M128b, M128a, -1.0)
            nc.tensor.matmul(s["S_ps"], M128a, M128b, start=False, stop=True)

            S = nsp.tile([P, P], F32, tag="S", name=f"S{g}")
            nc.vector.tensor_add(S, s["S_ps"], eps_eye)
            S2 = nsp.tile([P, P], F32, tag="S2", name=f"S2_{g}")
            frob2 = vecp.tile([P, 1], F32, name=f"fr{g}")
            nc.scalar.activation(
                S2, S, mybir.ActivationFunctionType.Square, accum_out=frob2
            )
            nc.gpsimd.partition_all_reduce(frob2, frob2, P, ReduceOp.add)
            nu = vecp.tile([P, 1], F32, name=f"nu{g}")
            nc.scalar.sqrt(nu, frob2)
            inv_nu = vecp.tile([P, 1], F32, name=f"inu{g}")
            nc.vector.reciprocal(inv_nu, nu)
            oscale = vecp.tile([P, 1], F32, name=f"osc{g}")
            nc.scalar.sqrt(oscale, inv_nu)
            s["oscale"] = oscale
            half_inv = vecp.tile([P, 1], F32, name=f"hin{g}")
            nc.vector.tensor_scalar_mul(half_inv, inv_nu, 0.5)
            S_half = nsp.tile([P, P], F16, tag="Sh", name=f"Sh{g}")
            nc.vector.tensor_scalar_mul(S_half, S, half_inv)
            s["S_half"] = S_half
            B = nsp.tile([P, P], F16, tag=f"B{g}", name=f"B0_{g}")
            nc.vector.tensor_sub(B, eye_15, S_half)
            s["B"] = B

        def emit_ns_step(g, it, sub):
            # one PE matmul of the NS chain + its trailing DVE op(s)
            s = st[g]
            if sub == 0:
                bb_ps = ps_ns.tile([P, P], F32, tag="nv", name=f"bb{g}_{it}")
                nc.tensor.matmul(bb_ps, s["B"], s["B"], start=True, stop=True)
                BB = nsp.tile([P, P], F16, tag=f"BB{g}", name=f"BB{g}_{it}")
                nc.vector.tensor_copy(BB, bb_ps)
                s["BB"] = BB
            elif sub == 1:
                b3_ps = ps_ns.tile([P, P], F32, tag="nv", name=f"b3{g}_{it}")
                nc.tensor.matmul(b3_ps, s["BB"], s["B"], start=True, stop=True)
                B3 = nsp.tile([P, P], F16, tag=f"B3{g}", name=f"B3_{g}_{it}")
                nc.vector.tensor_copy(B3, b3_ps)
                s["B3"] = B3
            else:
                p_ps = ps_ns.tile([P, P], F32, tag="nv", name=f"pp{g}_{it}")
                nc.tensor.matmul(p_ps, s["B3"], s["S_half"], start=True, stop=True)
                Bn = nsp.tile([P, P], F16, tag=f"Bn{g}", name=f"Bn{g}_{it}")
                nc.vector.tensor_scalar_mul(Bn, s["B"], 1.5)
                nc.vector.tensor_sub(Bn, Bn, p_ps)
                s["B"] = Bn

        def emit_cbias(g):
            s = st[g]
            c_ps = ps_vec.tile([P, 1], F32, tag="nv", name=f"cps{g}")
            nc.tensor.matmul(c_ps, s["B"], s["mean16"], start=True, stop=True)
            negos = vecp.tile([P, 1], F32, name=f"ng{g}")
            nc.vector.tensor_scalar_mul(negos, s["oscale"], -1.0)
            bias = vecp.tile([P, 1], F32, name=f"bi{g}")
            nc.vector.tensor_mul(bias, negos, c_ps)
            s["bias"] = bias

        def emit_proj_pair(g, pj, epi):
            # one 1024-wide projection unit: 2 matmuls into a [128,1024]
            # PSUM tile, ONE wide epilogue op (amortizes the ~300ns fixed
            # cost that capped 512-wide epilogues at ~265 GB/s of output
            # supply), then the 1024-wide store.
            #  epi 'act': ACT only (DVE pacing an NS chain); 'alt': rotate
            s = st[g]
            c, h = divmod(pj, 2)
            if h == 0:
                s["out_t"] = outp.tile([P, CHUNK], F32, tag="out", name=f"o{g}_{c}")
            pr = ps_pr.tile([P, 2 * SUB], F32, tag="pr", name=f"pr{g}_{pj}")
            for b in range(2):
                nc.tensor.matmul(
                    pr[:, ts(b, SUB)], s["B"],
                    s["zs"][c][:, ds(h * 2 * SUB + b * SUB, SUB)],
                    start=True, stop=True,
                )
            dst = s["out_t"][:, ds(h * 2 * SUB, 2 * SUB)]
            if epi == "act" or pj % 2 == 0:
                nc.scalar.activation(
                    dst, pr, mybir.ActivationFunctionType.Identity,
                    bias=s["bias"], scale=s["oscale"],
                )
            else:
                nc.vector.tensor_scalar(
                    dst, pr, s["oscale"], s["bias"],
                    mybir.AluOpType.mult, mybir.AluOpType.add,
                )
            nc.sync.dma_start(
                y[ds(g * P, P), ds(c * CHUNK + h * 2 * SUB, 2 * SUB)], dst
            )

        # ---------------- emission schedule ----------------
        emit_dma(0)
        emit_dma(1)
        for c in range(N_CHUNKS):
            emit_cast(0, c)
            for h in range(2):
                emit_gram_slice(0, 2 * c + h)

        # gram(1) chunk-by-chunk; its casts are arrival-paced with nothing
        # blocking ahead of them on ACT. mean(0)'s small ACT ops (frob
        # square + sqrts) are emitted after cast(1,2) so their deps are
        # already resolved when ACT reaches them; NS(0) follows.
        ns0 = [(it, sub) for it in range(T_NS - 1) for sub in range(3)]
        ns0_i = 0
        for c in range(N_CHUNKS):
            emit_cast(1, c)
            if c == 3:
                emit_mean_chain(0)
            for h in range(2):
                emit_gram_slice(1, 2 * c + h)
                # NS(0) paced at 2 steps per big-slice from chunk 4 on:
                # B(0) lands ~when the input stream ends, so output can
                # start immediately
                if c >= 4:
                    for _ in range(2):
                        if ns0_i < len(ns0):
                            it, sub = ns0[ns0_i]
                            emit_ns_step(0, it, sub)
                            ns0_i += 1
        while ns0_i < len(ns0):
            it, sub = ns0[ns0_i]
            emit_ns_step(0, it, sub)
            ns0_i += 1
        # bias(0) as soon as B(0) exists so g0 output can start early
        emit_cbias(0)
        emit_mean_chain(1)

        # NS(g1) interleaved with proj(0) at full two-engine epilogue rate.
        # The NS(1) chain stretches behind the DVE epilogues, but its
        # deadline is out1's bus window (~25us of slack) -- let it crawl.
        N_PAIRS = N_SLICES // 2
        p0 = 0
        for it in range(T_NS - 1):
            for sub in range(3):
                emit_ns_step(1, it, sub)
                for _ in range(2):
                    if p0 < N_PAIRS:
                        emit_proj_pair(0, p0, epi="alt")
                        p0 += 1
        emit_cbias(1)
        while p0 < N_PAIRS:
            emit_proj_pair(0, p0, epi="alt")
            p0 += 1
        for pj in range(N_PAIRS):
            emit_proj_pair(1, pj, epi="alt")

    nc.finalize()
    return nc


_NC_CACHE = None


def _get_nc():
    global _NC_CACHE
    if _NC_CACHE is None:
        _NC_CACHE = build_nc()
    return _NC_CACHE


def kernel(weight, _trace=False):
    w = np.ascontiguousarray(np.asarray(weight, dtype=np.float32))
    assert w.shape == (G_TOTAL * P, K), w.shape
    nc = _get_nc()
    in_maps = [
        {"x": np.ascontiguousarray(w[core * ROWS_PER_CORE:(core + 1) * ROWS_PER_CORE])}
        for core in range(N_CORES)
    ]
    res = run_bass_kernel_spmd(
        nc, in_maps, core_ids=list(range(N_CORES)), trace=_trace
    )
    out = np.concatenate([r["y"] for r in res.results], axis=0)
    if _trace:
        return out, res
    return out


# revision 47
# speedup vs baseline: 1.0573x; 1.0573x over previous
"""ONI-Norm TRN2 kernel: all-fp16 PE path, streamed to the HBM roofline.

The v4 baseline ran every matmul in fp32 (4 cyc/row on the PE) and was
PE-bound at ~188us (PE 171us active). This version is DMA-bound (~122us
for 37.75 MB in+out per core at the sustained ~350 GB/s the HAM power
duty-cycle allows):
  - Z fp32->fp16 cast on ACT per chunk, with accum_out yielding the
    row-sum partials for free (no DVE reduces; GpSimd casts measured
    3.4ns/elem -- useless for bulk work).
  - PE transposes read fp16 (1 cyc/row), gram + Newton-Schulz +
    projection matmuls all fp16 with fp32 PSUM accumulation.
  - transpose PSUM->SBUF copies on DVE; mean correction of the gram and
    the projection bias/scale keep everything on uncentered Z so the
    whole pipeline streams chunk-by-chunk behind the input DMA.
  - 1024-wide projection epilogues ([128,1024] PSUM tiles) alternating
    ACT/DVE: amortizes the ~300ns fixed per-op cost that otherwise caps
    output supply at ~265 GB/s.
  - emission order == execution-time order everywhere (in-order engine
    queues; anything queued out of order head-of-line blocks its engine).
Numerics validated in sim_v5.py: rel_max ~1e-3 vs the 2e-2 gate.
"""

import math
from contextlib import ExitStack

import numpy as np

import concourse.bacc as bacc
import concourse.mybir as mybir
from concourse.bass import ds, ts, MemorySpace
from concourse.bass_isa import ReduceOp
from concourse.bass_utils import run_bass_kernel_spmd
from concourse.masks import make_identity
from concourse.tile import TileContext

P = 128
K = 18432
G_TOTAL = 16
N_CORES = 8
G_PER_CORE = G_TOTAL // N_CORES
ROWS_PER_CORE = G_PER_CORE * P
T_NS = 5
EPS = 1e-5
CHUNK = 2048
N_CHUNKS = K // CHUNK
SUB = 512
SUB_PER_CHUNK = CHUNK // SUB
N_SLICES = N_CHUNKS * SUB_PER_CHUNK   # 36 per group
F32 = mybir.dt.float32
F16 = mybir.dt.float16


def build_nc():
    nc = bacc.Bacc("TRN2", target_bir_lowering=False)
    x = nc.dram_tensor("x", [ROWS_PER_CORE, K], F32, kind="ExternalInput")
    y = nc.dram_tensor("y", [ROWS_PER_CORE, K], F32, kind="ExternalOutput")

    with TileContext(nc) as tc, ExitStack() as ctx:
        consts = ctx.enter_context(tc.tile_pool(name="consts", bufs=1))
        identity = consts.tile([P, P], F32)
        make_identity(nc, identity)
        identity16 = consts.tile([P, P], F16)
        make_identity(nc, identity16)
        eye_15 = consts.tile([P, P], F16)
        nc.vector.tensor_scalar_mul(eye_15, identity, 1.5)
        eps_eye = consts.tile([P, P], F32)
        nc.vector.tensor_scalar_mul(eps_eye, identity, EPS)
        ones = consts.tile([P, P], F32)
        nc.any.memset(ones, 1.0)

        # fp32 landing chunks (rotating) and persistent fp16 copies
        zfp = ctx.enter_context(tc.tile_pool(name="zf", bufs=6))
        zpool = ctx.enter_context(tc.tile_pool(name="z", bufs=G_PER_CORE * N_CHUNKS))
        ztp = ctx.enter_context(tc.tile_pool(name="zt", bufs=3))
        outp = ctx.enter_context(tc.tile_pool(name="out", bufs=8))
        nsp = ctx.enter_context(tc.tile_pool(name="ns", bufs=2))
        vecp = ctx.enter_context(tc.tile_pool(name="vec", bufs=2))
        # PSUM banks: S(1) + tp(2) + pr(2x2 for 1024-wide tiles) + nv(1) = 8
        ps_S = ctx.enter_context(tc.tile_pool(name="psS", bufs=1, space=MemorySpace.PSUM))
        ps_tp = ctx.enter_context(tc.tile_pool(name="psT", bufs=2, space=MemorySpace.PSUM))
        ps_pr = ctx.enter_context(tc.tile_pool(name="psP", bufs=2, space=MemorySpace.PSUM))
        ps_nv = ctx.enter_context(tc.tile_pool(name="psN", bufs=1, space=MemorySpace.PSUM))
        ps_ns = ps_nv
        ps_vec = ps_nv

        st = [dict() for _ in range(G_PER_CORE)]

        def emit_dma(g):
            # all input DMA triggers upfront on the SP queue; full-size
            # 1MB transfers (a 512-split warm-up start was measured to
            # slow the input head ramp for no benefit -- PE has slack)
            s = st[g]
            s["zs"] = []
            s["zf"] = []
            s["rsum_parts"] = vecp.tile([P, N_CHUNKS], F32, name=f"rsp{g}")
            for c in range(N_CHUNKS):
                zf = zfp.tile([P, CHUNK], F32, tag="zf", name=f"zf{g}_{c}")
                z16 = zpool.tile([P, CHUNK], F16, tag="z", name=f"z{g}_{c}")
                nc.sync.dma_start(zf, x[ds(g * P, P), ts(c, CHUNK)])
                s["zf"].append(zf)
                s["zs"].append(z16)

        def emit_cast(g, c):
            # fp32->fp16 cast + fused row-sum partial in one ACT pass.
            # Emitted chunk-by-chunk so nothing with unresolved deps ever
            # sits ahead of a cast in the in-order ACT queue.
            s = st[g]
            nc.scalar.activation(
                s["zs"][c], s["zf"][c],
                mybir.ActivationFunctionType.Identity,
                accum_out=s["rsum_parts"][:, ds(c, 1)],
            )

        # gram works in 1024-wide big-slices: 8 transposes land in one
        # [128,1024] fp16 PSUM tile (exactly one bank), ONE DVE copy moves
        # it to SBUF (half the per-op fixed cost + semaphores of 512-wide)
        BIG = 2 * SUB
        N_BSLICES = K // BIG  # 18 per group, 2 per chunk

        def emit_gram_T(g, bsi):
            s = st[g]
            c, h = divmod(bsi, 2)
            tp = ps_tp.tile([P, BIG], F16, tag="tp", name=f"tp{g}_{bsi}")
            for b in range(BIG // P):
                nc.tensor.transpose(
                    tp[:, ts(b, P)],
                    s["zs"][c][:, ds(h * BIG + b * P, P)],
                    identity16,
                )
            zt = ztp.tile([P, BIG], F16, tag="zt", name=f"zt{g}_{bsi}")
            nc.vector.tensor_copy(zt, tp)
            s.setdefault("zt_pend", {})[bsi] = zt

        def emit_gram_M(g, bsi):
            s = st[g]
            if bsi == 0:
                s["S_ps"] = ps_S.tile([P, P], F32, tag="S", name=f"Sps{g}")
            zt = s["zt_pend"].pop(bsi)
            for b in range(BIG // P):
                nc.tensor.matmul(
                    s["S_ps"], zt[:, ts(b, P)], zt[:, ts(b, P)],
                    start=(bsi == 0 and b == 0), stop=False,
                )

        def emit_gram_slice(g, bsi):
            # transposes of big-slice bsi, then matmuls of bsi-1 (1 lag)
            emit_gram_T(g, bsi)
            if bsi > 0:
                emit_gram_M(g, bsi - 1)
            if bsi == N_BSLICES - 1:
                emit_gram_M(g, bsi)

        def emit_mean_chain(g):
            s = st[g]
            rsum = vecp.tile([P, 1], F32, name=f"rs{g}")
            nc.vector.tensor_reduce(
                rsum, s["rsum_parts"], mybir.AxisListType.X, mybir.AluOpType.add
            )
            mean = vecp.tile([P, 1], F32, name=f"mean{g}")
            nc.vector.tensor_scalar_mul(mean, rsum, 1.0 / K)
            mean16 = vecp.tile([P, 1], F16, name=f"mean16_{g}")
            nc.vector.tensor_copy(mean16, mean)
            s["mean16"] = mean16
            m12 = vecp.tile([P, 1], F32, name=f"m12{g}")
            nc.vector.tensor_scalar_mul(m12, rsum, math.sqrt(K / P) / K)
            Mm = vecp.tile([P, P], F32, name=f"Mm{g}")
            nc.vector.tensor_scalar_mul(Mm, ones, m12)
            M_ps = ps_vec.tile([P, P], F32, tag="nv", name=f"Mps{g}")
            nc.tensor.matmul(M_ps, Mm, identity, start=True, stop=True)
            M128a = vecp.tile([P, P], F32, name=f"Ma{g}")
            nc.vector.tensor_copy(M128a, M_ps)
            M128b = vecp.tile([P, P], F32, name=f"Mb{g}")
            nc.vector.tensor_scalar_mul(M128b, M128a, -1.0)
            nc.tensor.matmul(s["S_ps"], M128a, M128b, start=False, stop=True)

            S = nsp.tile([P, P], F32, tag="S", name=f"S{g}")
            nc.vector.tensor_add(S, s["S_ps"], eps_eye)
            S2 = nsp.tile([P, P], F32, tag="S2", name=f"S2_{g}")
            frob2 = vecp.tile([P, 1], F32, name=f"fr{g}")
            nc.scalar.activation(
                S2, S, mybir.ActivationFunctionType.Square, accum_out=frob2
            )
            nc.gpsimd.partition_all_reduce(frob2, frob2, P, ReduceOp.add)
            nu = vecp.tile([P, 1], F32, name=f"nu{g}")
            nc.scalar.sqrt(nu, frob2)
            inv_nu = vecp.tile([P, 1], F32, name=f"inu{g}")
            nc.vector.reciprocal(inv_nu, nu)
            oscale = vecp.tile([P, 1], F32, name=f"osc{g}")
            nc.scalar.sqrt(oscale, inv_nu)
            s["oscale"] = oscale
            half_inv = vecp.tile([P, 1], F32, name=f"hin{g}")
            nc.vector.tensor_scalar_mul(half_inv, inv_nu, 0.5)
            S_half = nsp.tile([P, P], F16, tag="Sh", name=f"Sh{g}")
            nc.vector.tensor_scalar_mul(S_half, S, half_inv)
            s["S_half"] = S_half
            B = nsp.tile([P, P], F16, tag=f"B{g}", name=f"B0_{g}")
            nc.vector.tensor_sub(B, eye_15, S_half)
            s["B"] = B

        def emit_ns_step(g, it, sub):
            # one PE matmul of the NS chain + its trailing DVE op(s)
            s = st[g]
            if sub == 0:
                bb_ps = ps_ns.tile([P, P], F32, tag="nv", name=f"bb{g}_{it}")
                nc.tensor.matmul(bb_ps, s["B"], s["B"], start=True, stop=True)
                BB = nsp.tile([P, P], F16, tag=f"BB{g}", name=f"BB{g}_{it}")
                nc.vector.tensor_copy(BB, bb_ps)
                s["BB"] = BB
            elif sub == 1:
                b3_ps = ps_ns.tile([P, P], F32, tag="nv", name=f"b3{g}_{it}")
                nc.tensor.matmul(b3_ps, s["BB"], s["B"], start=True, stop=True)
                B3 = nsp.tile([P, P], F16, tag=f"B3{g}", name=f"B3_{g}_{it}")
                nc.vector.tensor_copy(B3, b3_ps)
                s["B3"] = B3
            else:
                p_ps = ps_ns.tile([P, P], F32, tag="nv", name=f"pp{g}_{it}")
                nc.tensor.matmul(p_ps, s["B3"], s["S_half"], start=True, stop=True)
                Bn = nsp.tile([P, P], F16, tag=f"Bn{g}", name=f"Bn{g}_{it}")
                nc.vector.tensor_scalar_mul(Bn, s["B"], 1.5)
                nc.vector.tensor_sub(Bn, Bn, p_ps)
                s["B"] = Bn

        def emit_cbias(g):
            s = st[g]
            c_ps = ps_vec.tile([P, 1], F32, tag="nv", name=f"cps{g}")
            nc.tensor.matmul(c_ps, s["B"], s["mean16"], start=True, stop=True)
            negos = vecp.tile([P, 1], F32, name=f"ng{g}")
            nc.vector.tensor_scalar_mul(negos, s["oscale"], -1.0)
            bias = vecp.tile([P, 1], F32, name=f"bi{g}")
            nc.vector.tensor_mul(bias, negos, c_ps)
            s["bias"] = bias

        def emit_proj_pair(g, pj, epi):
            # one 1024-wide projection unit: 2 matmuls into a [128,1024]
            # PSUM tile, ONE wide epilogue op (amortizes the ~300ns fixed
            # cost that capped 512-wide epilogues at ~265 GB/s of output
            # supply), then the 1024-wide store.
            #  epi 'act': ACT only (DVE pacing an NS chain); 'alt': rotate
            s = st[g]
            c, h = divmod(pj, 2)
            if h == 0:
                s["out_t"] = outp.tile([P, CHUNK], F32, tag="out", name=f"o{g}_{c}")
            pr = ps_pr.tile([P, 2 * SUB], F32, tag="pr", name=f"pr{g}_{pj}")
            for b in range(2):
                nc.tensor.matmul(
                    pr[:, ts(b, SUB)], s["B"],
                    s["zs"][c][:, ds(h * 2 * SUB + b * SUB, SUB)],
                    start=True, stop=True,
                )
            dst = s["out_t"][:, ds(h * 2 * SUB, 2 * SUB)]
            if epi == "act" or pj % 2 == 0:
                nc.scalar.activation(
                    dst, pr, mybir.ActivationFunctionType.Identity,
                    bias=s["bias"], scale=s["oscale"],
                )
            else:
                nc.vector.tensor_scalar(
                    dst, pr, s["oscale"], s["bias"],
                    mybir.AluOpType.mult, mybir.AluOpType.add,
                )
            # full-chunk store: the two pair-epilogues run on different
            # engines concurrently, so waiting for both costs no latency
            if h == 1:
                nc.sync.dma_start(y[ds(g * P, P), ts(c, CHUNK)], s["out_t"])

        # ---------------- emission schedule ----------------
        emit_dma(0)
        emit_dma(1)
        for c in range(N_CHUNKS):
            emit_cast(0, c)
            for h in range(2):
                emit_gram_slice(0, 2 * c + h)

        # gram(1) chunk-by-chunk; its casts are arrival-paced with nothing
        # blocking ahead of them on ACT. mean(0)'s small ACT ops (frob
        # square + sqrts) are emitted after cast(1,2) so their deps are
        # already resolved when ACT reaches them; NS(0) follows.
        ns0 = [(it, sub) for it in range(T_NS - 1) for sub in range(3)]
        ns0_i = 0
        for c in range(N_CHUNKS):
            emit_cast(1, c)
            if c == 3:
                emit_mean_chain(0)
            for h in range(2):
                emit_gram_slice(1, 2 * c + h)
                # NS(0) paced at 2 steps per big-slice from chunk 4 on:
                # B(0) lands ~when the input stream ends, so output can
                # start immediately
                if c >= 4:
                    for _ in range(2):
                        if ns0_i < len(ns0):
                            it, sub = ns0[ns0_i]
                            emit_ns_step(0, it, sub)
                            ns0_i += 1
        while ns0_i < len(ns0):
            it, sub = ns0[ns0_i]
            emit_ns_step(0, it, sub)
            ns0_i += 1
        # bias(0) as soon as B(0) exists so g0 output can start early
        emit_cbias(0)
        emit_mean_chain(1)

        # NS(g1) interleaved with proj(0) at full two-engine epilogue rate.
        # The NS(1) chain stretches behind the DVE epilogues, but its
        # deadline is out1's bus window (~25us of slack) -- let it crawl.
        N_PAIRS = N_SLICES // 2
        p0 = 0
        for it in range(T_NS - 1):
            for sub in range(3):
                emit_ns_step(1, it, sub)
                for _ in range(2):
                    if p0 < N_PAIRS:
                        emit_proj_pair(0, p0, epi="alt")
                        p0 += 1
        emit_cbias(1)
        while p0 < N_PAIRS:
            emit_proj_pair(0, p0, epi="alt")
            p0 += 1
        for pj in range(N_PAIRS):
            emit_proj_pair(1, pj, epi="alt")

    nc.finalize()
    return nc


_NC_CACHE = None


def _get_nc():
    global _NC_CACHE
    if _NC_CACHE is None:
        _NC_CACHE = build_nc()
    return _NC_CACHE


def kernel(weight, _trace=False):
    w = np.ascontiguousarray(np.asarray(weight, dtype=np.float32))
    assert w.shape == (G_TOTAL * P, K), w.shape
    nc = _get_nc()
    in_maps = [
        {"x": np.ascontiguousarray(w[core * ROWS_PER_CORE:(core + 1) * ROWS_PER_CORE])}
        for core in range(N_CORES)
    ]
    res = run_bass_kernel_spmd(
        nc, in_maps, core_ids=list(range(N_CORES)), trace=_trace
    )
    out = np.concatenate([r["y"] for r in res.results], axis=0)
    if _trace:
        return out, res
    return out
